# revision 1
# baseline (speedup 1.0000x reference)
"""GNN message-passing (std aggregator) on 8 TRN2 NeuronCores.

Math per target node: count, S1 = sum x[src], S2 = sum x[src]^2;
mean = S1/max(count,eps); var = S2/count - mean^2;
std = sqrt(max(var,0)), zeroed where count <= 1.

Strategy: shard TARGET nodes across cores (no collectives). Host packs nodes
into 128-bin blocks balanced by in-degree (serpentine deal), buckets edges by
(block, src-quarter) with uniform tile capacity tq per (block,quarter) so one
NEFF serves all cores. Device per core, per group of GB blocks:
  - 4x dma_gather (one per src quarter of x; int16 idx < 25000) pulls
    x[src] rows into SBUF in quarter-major column order,
  - ACT builds rhs tiles [x | x^2 | 1] (cast to MM dtype),
  - DVE builds 4-wide one-hot tiles (slot-vs-iota is_equal),
  - PE matmul-accumulates [128 bins x 129] = [S1 | S2 | count] in PSUM,
  - small DVE/ACT finishing pass computes std, DMA out per block.
"""

import numpy as np

N_NODES = 100000
N_FEAT = 64
N_EDGES = 1600000
P = 128
NCORES = 8
NB = 98                 # blocks per core
NBLK = NCORES * NB      # 784
GB = 7                  # blocks per group; 98 = 14*7
NQUART = 4
NQ = N_NODES // NQUART  # rows per src quarter (25000 < 32768 for int16 idx)
EPS = 1e-8
MM_DT = "bfloat16"      # "float32" | "bfloat16" for matmul operands

_CACHE = {}


def _build_program(n_nodes, f, nb, tq, gb, nq, mm_dt):
    import concourse.bass as bass
    import concourse.bacc as bacc
    import concourse.mybir as mybir
    import concourse.tile as tile

    F32 = mybir.dt.float32
    I16 = mybir.dt.int16
    MDT = getattr(mybir.dt, mm_dt)
    AO = mybir.AluOpType
    AF = mybir.ActivationFunctionType

    t = NQUART * tq            # tiles (columns) per block
    W = 2 * f + 1              # 129
    C = nb * t                 # total columns per core
    gcols = gb * t             # columns per group
    qcols = gb * tq            # columns per (group, quarter)
    ng = nb // gb
    nidx = qcols * P           # indices per gather
    i16c = nidx // 16          # idx16 cols per gather

    nc = bacc.Bacc()
    xd = nc.declare_dram_parameter("x", [n_nodes, f], F32, isOutput=False)
    gidxd = nc.declare_dram_parameter(
        "gidx", [P, ng * NQUART * i16c], I16, isOutput=False)
    tgtd = nc.declare_dram_parameter("tgt", [P, C], F32, isOutput=False)
    outd = nc.declare_dram_parameter("out", [nb * P, f], F32, isOutput=True)

    with tile.TileContext(nc) as tc:
        with (
            tc.tile_pool(name="const", bufs=1) as constp,
            tc.tile_pool(name="io", bufs=2) as iop,
            tc.tile_pool(name="msg", bufs=2) as msgp,
            tc.tile_pool(name="oh", bufs=6) as ohp,
            tc.tile_pool(name="fin", bufs=4) as finp,
            tc.tile_pool(name="ov", bufs=4) as ovp,
            tc.tile_pool(name="ps", bufs=8, space="PSUM") as psump,
        ):
            # 4-wide iota [128, 4*128]: value = column index % 128
            iota4 = constp.tile([P, 4 * P], F32)
            nc.gpsimd.iota(iota4[:], pattern=[[0, 4], [1, P]], base=0,
                           channel_multiplier=0,
                           allow_small_or_imprecise_dtypes=True)

            for g in range(ng):
                idx = iop.tile([P, NQUART * i16c], I16, tag="idx")
                tg = iop.tile([P, gcols], F32, tag="tg")
                nc.sync.dma_start(
                    out=idx[:],
                    in_=gidxd[:, g * NQUART * i16c:(g + 1) * NQUART * i16c])
                nc.sync.dma_start(
                    out=tg[:], in_=tgtd[:, g * gcols:(g + 1) * gcols])
                tgv = iop.tile([P, gcols], F32, tag="tgv")
                nc.vector.tensor_copy(out=tgv[:], in_=tg[:])

                gbuf = msgp.tile([P, gcols * f], F32, tag="g")
                g3 = gbuf[:].rearrange("p (c e) -> p c e", e=f)
                for qq in range(NQUART):
                    nc.gpsimd.dma_gather(
                        out_ap=g3[:, qq * qcols:(qq + 1) * qcols, :],
                        in_ap=xd[qq * nq:(qq + 1) * nq, :],
                        idxs_ap=idx[:, qq * i16c:(qq + 1) * i16c],
                        num_idxs=nidx,
                        num_idxs_reg=nidx,
                        elem_size=f,
                        single_packet=False,
                    )
                sqx = msgp.tile([P, gcols * W], MDT, tag="sqx")
                s3 = sqx[:].rearrange("p (c w) -> p c w", w=W)
                nc.scalar.activation(out=s3[:, :, 0:f], in_=g3[:, :, :],
                                     func=AF.Copy)
                nc.scalar.square(out=s3[:, :, f:2 * f], in_=g3[:, :, :])
                nc.scalar.activation(out=s3[:, :, 2 * f:W], in_=g3[:, :, 0:1],
                                     func=AF.Copy, bias=1.0, scale=0.0)

                pss = [psump.tile([P, W], F32, tag="ps", name=f"ps_{g}_{bl}")
                       for bl in range(gb)]
                for pk in range(gcols // 4):
                    oh4 = ohp.tile([P, 4 * P], MDT)
                    nc.vector.tensor_tensor(
                        out=oh4[:].rearrange("p (c e) -> p c e", e=P),
                        in0=tgv[:, 4 * pk:4 * pk + 4]
                            .rearrange("p (c u) -> p c u", u=1)
                            .to_broadcast([P, 4, P]),
                        in1=iota4[:].rearrange("p (c e) -> p c e", e=P),
                        op=AO.is_equal,
                    )
                    for i in range(4):
                        cl = 4 * pk + i
                        qq = cl // qcols
                        r = cl % qcols
                        bl = r // tq
                        j = r % tq
                        nc.tensor.matmul(
                            out=pss[bl][:],
                            lhsT=oh4[:, i * P:(i + 1) * P],
                            rhs=sqx[:, cl * W:(cl + 1) * W],
                            start=(qq == 0 and j == 0),
                            stop=(qq == NQUART - 1 and j == tq - 1),
                        )
                for bl in range(gb):
                    b = g * gb + bl
                    ps = pss[bl]
                    cnt = finp.tile([P, 1], F32, tag="cnt")
                    nc.vector.tensor_scalar(
                        out=cnt[:], in0=ps[:, 2 * f:W],
                        scalar1=float(EPS), scalar2=None, op0=AO.max)
                    rec = finp.tile([P, 1], F32, tag="rec")
                    nc.vector.reciprocal(out=rec[:], in_=cnt[:])
                    mean = finp.tile([P, f], F32, tag="mean")
                    nc.vector.tensor_scalar_mul(
                        out=mean[:], in0=ps[:, 0:f], scalar1=rec[:])
                    ex2 = finp.tile([P, f], F32, tag="ex2")
                    nc.vector.tensor_scalar_mul(
                        out=ex2[:], in0=ps[:, f:2 * f], scalar1=rec[:])
                    var = finp.tile([P, f], F32, tag="var")
                    nc.vector.tensor_tensor(
                        out=var[:], in0=mean[:], in1=mean[:], op=AO.mult)
                    nc.vector.tensor_tensor(
                        out=var[:], in0=ex2[:], in1=var[:], op=AO.subtract)
                    nc.vector.tensor_scalar(
                        out=var[:], in0=var[:], scalar1=0.0, scalar2=None,
                        op0=AO.max)
                    std = ovp.tile([P, f], F32, tag="std")
                    nc.scalar.sqrt(out=std[:], in_=var[:])
                    mask = finp.tile([P, 1], F32, tag="mask")
                    nc.vector.tensor_scalar(
                        out=mask[:], in0=ps[:, 2 * f:W],
                        scalar1=1.5, scalar2=None, op0=AO.is_gt)
                    nc.vector.tensor_scalar_mul(
                        out=std[:], in0=std[:], scalar1=mask[:])
                    nc.sync.dma_start(
                        out=outd[b * P:(b + 1) * P, :], in_=std[:])
    return nc


def _host_prep(x, edge_index):
    src = np.asarray(edge_index[0], dtype=np.int64)
    tgt = np.asarray(edge_index[1], dtype=np.int64)
    n_edges = src.shape[0]
    counts = np.bincount(tgt, minlength=N_NODES)

    # serpentine deal of count-sorted nodes into NBLK blocks of <=128 slots
    order = np.argsort(-counts, kind="stable")
    ranks = np.arange(N_NODES)
    rounds = ranks // NBLK
    pos = ranks % NBLK
    blk_of_rank = np.where(rounds % 2 == 0, pos, NBLK - 1 - pos)
    blk = np.empty(N_NODES, np.int64)
    slot = np.empty(N_NODES, np.int64)
    blk[order] = blk_of_rank
    slot[order] = rounds
    assert slot.max() < P

    eb = blk[tgt]                      # edge -> block
    eq = src // NQ                     # edge -> src quarter
    es = slot[tgt]                     # edge -> slot in block
    seg = eb * NQUART + eq             # edge -> (block, quarter) segment
    segsums = np.bincount(seg, minlength=NBLK * NQUART)
    tq = int(np.ceil(segsums.max() / P))
    cap = tq * P

    order_e = np.argsort(seg, kind="stable")
    segs = seg[order_e]
    starts = np.zeros(NBLK * NQUART, np.int64)
    np.cumsum(segsums[:-1], out=starts[1:])
    within = np.arange(n_edges) - starts[segs]
    flat = segs * cap + within

    gidxq = np.zeros((NBLK, NQUART, cap), np.int16)
    tgtq = np.full((NBLK, NQUART, cap), -1.0, np.float32)
    gidxq.reshape(-1)[flat] = (src[order_e] % NQ).astype(np.int16)
    tgtq.reshape(-1)[flat] = es[order_e].astype(np.float32)

    xf = np.ascontiguousarray(np.asarray(x, dtype=np.float32))
    ng = NB // GB
    i16c = GB * cap // 16

    in_maps = []
    for c in range(NCORES):
        tb = tgtq[c * NB:(c + 1) * NB]          # [NB, 4, cap]
        gi = gidxq[c * NB:(c + 1) * NB]
        # tgt columns: (group, quarter, block, tile) -> [P, C]
        tcore = (tb.reshape(ng, GB, NQUART, cap)
                 .transpose(0, 2, 1, 3)          # [ng, 4, GB, cap]
                 .reshape(ng * NQUART * GB * tq, P).T)
        # idx16: per (group, quarter): stream of GB*cap idxs wrapped %16
        gs = (gi.reshape(ng, GB, NQUART, cap)
              .transpose(0, 2, 1, 3)             # [ng, 4, GB, cap]
              .reshape(ng * NQUART, GB * cap))   # per-gather streams
        idx16 = np.ascontiguousarray(
            np.tile(gs.reshape(ng * NQUART, i16c, 16).transpose(0, 2, 1)
                    .reshape(ng * NQUART * 16, i16c)
                    .reshape(ng * NQUART, 16, i16c)
                    .transpose(1, 0, 2).reshape(16, ng * NQUART * i16c),
                    (8, 1)))
        in_maps.append({
            "x": xf,
            "gidx": idx16,
            "tgt": np.ascontiguousarray(tcore),
        })
    return tq, in_maps, blk, slot


def _run(x, edge_index, trace=False):
    from concourse.bass_utils import run_bass_kernel_spmd

    tq, in_maps, blk, slot = _host_prep(x, edge_index)
    key = ("prog", tq, MM_DT)
    if key not in _CACHE:
        nc_ = _build_program(N_NODES, N_FEAT, NB, tq, GB, NQ, MM_DT)
        nc_.finalize()
        _CACHE[key] = nc_
    nc = _CACHE[key]
    res = run_bass_kernel_spmd(
        nc, in_maps, core_ids=list(range(NCORES)), trace=trace)

    outs = [np.asarray(r["out"]) for r in res.results]
    out_full = np.empty((N_NODES, N_FEAT), np.float32)
    cores = blk // NB
    rows = (blk % NB) * P + slot
    for c in range(NCORES):
        m = cores == c
        out_full[m] = outs[c][rows[m]]
    return out_full, res


def kernel(**inputs):
    out, _ = _run(inputs["x"], inputs["edge_index"], trace=False)
    return out



# revision 6
# speedup vs baseline: 2.9698x; 2.9698x over previous
"""GNN message-passing (std aggregator) on 8 TRN2 NeuronCores.

Math per target node: count, S1 = sum x[src], S2 = sum x[src]^2;
mean = S1/max(count,eps); var = S2/count - mean^2;
std = sqrt(max(var,0)), zeroed where count <= 1.

Strategy: shard TARGET nodes across cores (no collectives). Host packs nodes
into 128-bin blocks balanced by in-degree (serpentine deal). Edges bucketed by
(block, src-quarter); each (block, quarter) segment gets its own dma_gather
(idx16 < 25000) with the real indices first and a NEGATIVE tail — the gather
ucode trims trailing negatives, so padding costs zero descriptor time. Gathers
round-robin across 4 SWDGE queues so up to 4 GpSimd Q7 core-pairs generate
descriptors concurrently (~3x). Per group of GB blocks:
  - ACT builds rhs tiles [x | x^2 | 1] (cast to bf16),
  - DVE builds per-block one-hot tiles (slot-vs-iota is_equal),
  - PE matmul-accumulates [128 bins x 129] = [S1 | S2 | count] in PSUM,
  - batched finishing pass (ACT psum drain + wide DVE ops) computes std,
    one strided DMA out per group.
"""

import numpy as np

N_NODES = 100000
N_FEAT = 64
N_EDGES = 1600000
P = 128
NCORES = 8
NB = 98                 # blocks per core
NBLK = NCORES * NB      # 784
GB = 7                  # blocks per group; 98 = 14*7
NQUART = 4
NQ = N_NODES // NQUART  # rows per src quarter (25000 < 32768 for int16 idx)
EPS = 1e-8
MM_DT = "bfloat16"      # matmul operand dtype

_CACHE = {}


def _build_program(n_nodes, f, nb, tq, gb, nq, mm_dt):
    import concourse.bass as bass
    import concourse.bacc as bacc
    import concourse.mybir as mybir
    import concourse.tile as tile

    F32 = mybir.dt.float32
    I16 = mybir.dt.int16
    MDT = getattr(mybir.dt, mm_dt)
    AO = mybir.AluOpType
    AF = mybir.ActivationFunctionType

    cap = tq * P               # idx slots per (block, quarter) gather
    tpb = NQUART * tq          # tile-columns per block
    W = 2 * f + 1              # 129
    C = nb * tpb               # total columns per core
    gcols = gb * tpb           # columns per group
    ng = nb // gb
    i16s = cap // 16           # idx16 cols per (block, quarter) gather

    nc = bacc.Bacc(num_swdge_queues=4)
    xd = nc.declare_dram_parameter("x", [n_nodes, f], F32, isOutput=False)
    gidxd = nc.declare_dram_parameter(
        "gidx", [P, nb * NQUART * i16s], I16, isOutput=False)
    tgtd = nc.declare_dram_parameter("tgt", [P, C], F32, isOutput=False)
    outd = nc.declare_dram_parameter("out", [nb * P, f], F32, isOutput=True)

    with tile.TileContext(nc) as tc:
        with (
            tc.tile_pool(name="const", bufs=1) as constp,
            tc.tile_pool(name="io", bufs=2) as iop,
            tc.tile_pool(name="msg", bufs=2) as msgp,
            tc.tile_pool(name="oh", bufs=3) as ohp,
            tc.tile_pool(name="fin", bufs=2) as finp,
            tc.tile_pool(name="ov", bufs=2) as ovp,
            tc.tile_pool(name="ps", bufs=8, space="PSUM") as psump,
        ):
            # per-block iota [128, tpb*128]: value = column index % 128
            iotat = constp.tile([P, tpb * P], F32)
            nc.gpsimd.iota(iotat[:], pattern=[[0, tpb], [1, P]], base=0,
                           channel_multiplier=0,
                           allow_small_or_imprecise_dtypes=True)

            # zero both message buffers once: trimmed gather tails leave
            # stale SBUF, which must be finite (0*x in the matmul).
            for _ in range(2):
                gz = msgp.tile([P, gcols * f], F32, tag="g")
                nc.vector.memset(gz[:], 0.0)

            gseq = 0
            out3 = outd[:].rearrange("(b p) f -> p b f", p=P)
            for g in range(ng):
                idx = iop.tile([P, gb * NQUART * i16s], I16, tag="idx")
                nc.sync.dma_start(
                    out=idx[:],
                    in_=gidxd[:, g * gb * NQUART * i16s:
                              (g + 1) * gb * NQUART * i16s])
                tg = iop.tile([P, gcols], F32, tag="tg")
                nc.sync.dma_start(
                    out=tg[:], in_=tgtd[:, g * gcols:(g + 1) * gcols])

                gbuf = msgp.tile([P, gcols * f], F32, tag="g")
                g3 = gbuf[:].rearrange("p (c e) -> p c e", e=f)
                for bl in range(gb):
                    for q in range(NQUART):
                        c0 = bl * tpb + q * tq
                        nc.gpsimd.dma_gather(
                            out_ap=g3[:, c0:c0 + tq, :],
                            in_ap=xd[q * nq:(q + 1) * nq, :],
                            idxs_ap=idx[:, (bl * NQUART + q) * i16s:
                                        (bl * NQUART + q + 1) * i16s],
                            num_idxs=cap,
                            num_idxs_reg=cap,
                            elem_size=f,
                            single_packet=False,
                            queue_num=gseq % 4,
                        )
                        gseq += 1

                sqx = msgp.tile([P, gcols * W], MDT, tag="sqx")
                s3 = sqx[:].rearrange("p (c w) -> p c w", w=W)
                nc.scalar.activation(out=s3[:, :, 0:f], in_=g3[:, :, :],
                                     func=AF.Copy)
                nc.scalar.square(out=s3[:, :, f:2 * f], in_=g3[:, :, :])
                nc.scalar.activation(out=s3[:, :, 2 * f:W], in_=g3[:, :, 0:1],
                                     func=AF.Copy, bias=1.0, scale=0.0)

                pss = [psump.tile([P, W], F32, tag="ps", name=f"ps_{g}_{bl}")
                       for bl in range(gb)]
                for bl in range(gb):
                    oh = ohp.tile([P, tpb * P], MDT)
                    nc.vector.tensor_tensor(
                        out=oh[:].rearrange("p (c e) -> p c e", e=P),
                        in0=tg[:, bl * tpb:(bl + 1) * tpb]
                            .rearrange("p (c u) -> p c u", u=1)
                            .to_broadcast([P, tpb, P]),
                        in1=iotat[:].rearrange("p (c e) -> p c e", e=P),
                        op=AO.is_equal,
                    )
                    for t in range(tpb):
                        cl = bl * tpb + t
                        nc.tensor.matmul(
                            out=pss[bl][:],
                            lhsT=oh[:, t * P:(t + 1) * P],
                            rhs=sqx[:, cl * W:(cl + 1) * W],
                            start=(t == 0),
                            stop=(t == tpb - 1),
                        )

                # batched finishing pass over the group's gb blocks
                fin = finp.tile([P, gb * W], F32, tag="fin")
                for bl in range(gb):
                    nc.scalar.activation(out=fin[:, bl * W:(bl + 1) * W],
                                         in_=pss[bl][:], func=AF.Copy)
                f3 = fin[:].rearrange("p (b w) -> p b w", w=W)
                cnt = finp.tile([P, gb], F32, tag="cnt")
                nc.vector.tensor_scalar(
                    out=cnt[:].rearrange("p (b u) -> p b u", u=1),
                    in0=f3[:, :, 2 * f:2 * f + 1],
                    scalar1=float(EPS), scalar2=None, op0=AO.max)
                rec = finp.tile([P, gb], F32, tag="rec")
                nc.vector.reciprocal(out=rec[:], in_=cnt[:])
                r3 = rec[:].rearrange("p (b u) -> p b u", u=1)
                mom = finp.tile([P, gb * 2 * f], F32, tag="mom")
                m3 = mom[:].rearrange("p (b w) -> p b w", w=2 * f)
                nc.vector.tensor_tensor(
                    out=m3[:, :, :], in0=f3[:, :, 0:2 * f],
                    in1=r3.to_broadcast([P, gb, 2 * f]), op=AO.mult)
                var = finp.tile([P, gb * f], F32, tag="var")
                v3 = var[:].rearrange("p (b w) -> p b w", w=f)
                nc.vector.tensor_tensor(
                    out=v3[:, :, :], in0=m3[:, :, 0:f], in1=m3[:, :, 0:f],
                    op=AO.mult)
                nc.vector.tensor_tensor(
                    out=v3[:, :, :], in0=m3[:, :, f:2 * f], in1=v3[:, :, :],
                    op=AO.subtract)
                nc.vector.tensor_scalar(
                    out=var[:], in0=var[:], scalar1=0.0, scalar2=None,
                    op0=AO.max)
                std = ovp.tile([P, gb * f], F32, tag="std")
                nc.scalar.sqrt(out=std[:], in_=var[:])
                mask = finp.tile([P, gb], F32, tag="mask")
                nc.vector.tensor_scalar(
                    out=mask[:].rearrange("p (b u) -> p b u", u=1),
                    in0=f3[:, :, 2 * f:2 * f + 1],
                    scalar1=1.5, scalar2=None, op0=AO.is_gt)
                s3o = std[:].rearrange("p (b w) -> p b w", w=f)
                nc.vector.tensor_tensor(
                    out=s3o[:, :, :], in0=s3o[:, :, :],
                    in1=mask[:].rearrange("p (b u) -> p b u", u=1)
                        .to_broadcast([P, gb, f]),
                    op=AO.mult)
                nc.sync.dma_start(
                    out=out3[:, g * gb:(g + 1) * gb, :], in_=s3o[:, :, :])
    return nc


def _host_prep(x, edge_index):
    src = np.asarray(edge_index[0], dtype=np.int64)
    tgt = np.asarray(edge_index[1], dtype=np.int64)
    n_edges = src.shape[0]
    counts = np.bincount(tgt, minlength=N_NODES)

    # serpentine deal of count-sorted nodes into NBLK blocks of <=128 slots
    order = np.argsort(-counts, kind="stable")
    ranks = np.arange(N_NODES)
    rounds = ranks // NBLK
    pos = ranks % NBLK
    blk_of_rank = np.where(rounds % 2 == 0, pos, NBLK - 1 - pos)
    blk = np.empty(N_NODES, np.int64)
    slot = np.empty(N_NODES, np.int64)
    blk[order] = blk_of_rank
    slot[order] = rounds
    assert slot.max() < P

    eb = blk[tgt]                      # edge -> block
    eq = src // NQ                     # edge -> src quarter
    es = slot[tgt]                     # edge -> slot in block
    seg = eb * NQUART + eq             # edge -> (block, quarter) segment
    segsums = np.bincount(seg, minlength=NBLK * NQUART)
    tq = int(np.ceil(segsums.max() / P))
    cap = tq * P

    order_e = np.argsort(seg, kind="stable")
    segs = seg[order_e]
    starts = np.zeros(NBLK * NQUART, np.int64)
    np.cumsum(segsums[:-1], out=starts[1:])
    within = np.arange(n_edges) - starts[segs]
    flat = segs * cap + within

    # real idxs first, NEGATIVE tail (gather ucode trims trailing negatives)
    gidxq = np.zeros((NBLK, NQUART, cap), np.int16)  # BISECT: no trim
    tgtq = np.full((NBLK, NQUART, cap), -1.0, np.float32)
    gidxq.reshape(-1)[flat] = (src[order_e] % NQ).astype(np.int16)
    tgtq.reshape(-1)[flat] = es[order_e].astype(np.float32)

    xf = np.ascontiguousarray(np.asarray(x, dtype=np.float32))
    i16s = cap // 16

    in_maps = []
    for c in range(NCORES):
        tb = tgtq[c * NB:(c + 1) * NB]          # [NB, 4, cap]
        gi = gidxq[c * NB:(c + 1) * NB]
        # tgt columns: (block, quarter, tile) -> [P, C]
        tcore = np.ascontiguousarray(
            tb.reshape(NB, NQUART, tq, P)
            .transpose(3, 0, 1, 2)               # [P, NB, 4, tq]
            .reshape(P, NB * NQUART * tq))
        # idx16 per (block, quarter): cap idxs wrapped %16, replicated x8
        gi16 = (gi.reshape(NB * NQUART, i16s, 16)
                .transpose(0, 2, 1)              # [seg, 16, i16s]
                .reshape(NB * NQUART, 16, i16s)
                .transpose(1, 0, 2)              # [16, seg, i16s]
                .reshape(16, NB * NQUART * i16s))
        idx16 = np.ascontiguousarray(np.tile(gi16, (8, 1)))
        in_maps.append({
            "x": xf,
            "gidx": idx16,
            "tgt": tcore,
        })
    return tq, in_maps, blk, slot


def _run(x, edge_index, trace=False):
    from concourse.bass_utils import run_bass_kernel_spmd

    tq, in_maps, blk, slot = _host_prep(x, edge_index)
    key = ("prog", tq, MM_DT)
    if key not in _CACHE:
        nc_ = _build_program(N_NODES, N_FEAT, NB, tq, GB, NQ, MM_DT)
        nc_.finalize()
        _CACHE[key] = nc_
    nc = _CACHE[key]
    res = run_bass_kernel_spmd(
        nc, in_maps, core_ids=list(range(NCORES)), trace=trace)

    outs = [np.asarray(r["out"]) for r in res.results]
    out_full = np.empty((N_NODES, N_FEAT), np.float32)
    cores = blk // NB
    rows = (blk % NB) * P + slot
    for c in range(NCORES):
        m = cores == c
        out_full[m] = outs[c][rows[m]]
    return out_full, res


def kernel(**inputs):
    out, _ = _run(inputs["x"], inputs["edge_index"], trace=False)
    return out


# revision 8
# speedup vs baseline: 3.6725x; 1.2366x over previous
"""GNN message-passing (std aggregator) on 8 TRN2 NeuronCores.

Math per target node: count, S1 = sum x[src], S2 = sum x[src]^2;
mean = S1/max(count,eps); var = S2/count - mean^2;
std = sqrt(max(var,0)), zeroed where count <= 1.

Strategy: shard TARGET nodes across cores (no collectives). Host packs nodes
into 128-bin blocks with a greedy 4-dim balancer (per-quarter loads <= ~512),
sorts blocks by load and deals them serpentine to cores so every core has the
same per-position load profile. Each block position gets its own compile-time
capacity (128-multiple), so gather padding is ~2-3% instead of 25%. Per group
of GB blocks and src-quarter q there is ONE dma_gather (int16 idx < 25000);
gathers round-robin 4 SWDGE queues so 4 GpSimd Q7 pairs emit descriptors
concurrently (~3.2x). Per group: ACT builds [x | x^2 | 1] bf16 rhs, DVE builds
one-hot tiles (label-vs-iota is_equal), PE accumulates [128 x 129] = [S1 | S2
| count] per block in PSUM, then a batched finishing pass computes std and one
strided DMA per group writes out.
"""

import numpy as np

N_NODES = 100000
N_FEAT = 64
N_EDGES = 1600000
P = 128
NCORES = 8
NB = 98                 # blocks per core
NBLK = NCORES * NB      # 784
GB = 7                  # blocks per group; 98 = 14*7
NG = NB // GB
NQUART = 4
NQ = N_NODES // NQUART  # rows per src quarter (25000 < 32768 for int16 idx)
EPS = 1e-8
MM_DT = "bfloat16"      # matmul operand dtype

_CACHE = {}


def _build_program(caps, mm_dt):
    """caps: tuple of NB ints, capacity (multiple of 128) per block position."""
    import concourse.bacc as bacc
    import concourse.mybir as mybir
    import concourse.tile as tile

    F32 = mybir.dt.float32
    I16 = mybir.dt.int16
    MDT = getattr(mybir.dt, mm_dt)
    AO = mybir.AluOpType
    AF = mybir.ActivationFunctionType

    f = N_FEAT
    W = 2 * f + 1
    tiles = [c // P for c in caps]               # tile-columns per (pos, q)
    # per-group geometry
    gtiles = [sum(tiles[g * GB:(g + 1) * GB]) for g in range(NG)]  # per q
    gcols_g = [4 * t for t in gtiles]            # tile-cols per group
    maxgt = max(gtiles)
    maxgc = max(gcols_g)
    C = sum(gcols_g)                             # total columns per core
    i16_gq = [t * P // 16 for t in gtiles]       # idx16 cols per (g, q) gather
    IC = 4 * sum(i16_gq)                         # idx16 cols per core

    nc = bacc.Bacc(num_swdge_queues=4)
    xd = nc.declare_dram_parameter("x", [N_NODES, f], F32, isOutput=False)
    gidxd = nc.declare_dram_parameter("gidx", [P, IC], I16, isOutput=False)
    tgtd = nc.declare_dram_parameter("tgt", [P, C], F32, isOutput=False)
    outd = nc.declare_dram_parameter("out", [NB * P, f], F32, isOutput=True)

    with tile.TileContext(nc) as tc:
        with (
            tc.tile_pool(name="const", bufs=1) as constp,
            tc.tile_pool(name="io", bufs=2) as iop,
            tc.tile_pool(name="msg", bufs=2) as msgp,
            tc.tile_pool(name="oh", bufs=2) as ohp,
            tc.tile_pool(name="fin", bufs=2) as finp,
            tc.tile_pool(name="ov", bufs=2) as ovp,
            tc.tile_pool(name="ps", bufs=8, space="PSUM") as psump,
        ):
            iotat = constp.tile([P, maxgt * P], F32)
            nc.gpsimd.iota(iotat[:], pattern=[[0, maxgt], [1, P]], base=0,
                           channel_multiplier=0,
                           allow_small_or_imprecise_dtypes=True)

            out3 = outd[:].rearrange("(b p) f -> p b f", p=P)
            ioff = 0   # idx16 column offset
            coff = 0   # tgt column offset
            ooff = 0   # out block offset
            for g in range(NG):
                gt = gtiles[g]
                gc = gcols_g[g]
                i16g = i16_gq[g]
                idx = iop.tile([P, 4 * max(i16_gq)], I16, tag="idx")
                nc.sync.dma_start(out=idx[:, 0:4 * i16g],
                                  in_=gidxd[:, ioff:ioff + 4 * i16g])
                tg = iop.tile([P, maxgc], F32, tag="tg")
                nc.sync.dma_start(out=tg[:, 0:gc],
                                  in_=tgtd[:, coff:coff + gc])

                gbuf = msgp.tile([P, maxgc * f], F32, tag="g")
                g3 = gbuf[:].rearrange("p (c e) -> p c e", e=f)
                for q in range(NQUART):
                    nc.gpsimd.dma_gather(
                        out_ap=g3[:, q * gt:(q + 1) * gt, :],
                        in_ap=xd[q * NQ:(q + 1) * NQ, :],
                        idxs_ap=idx[:, q * i16g:(q + 1) * i16g],
                        num_idxs=gt * P,
                        num_idxs_reg=gt * P,
                        elem_size=f,
                        single_packet=False,
                        queue_num=q,
                    )

                sqx = msgp.tile([P, maxgc * W], MDT, tag="sqx")
                s3 = sqx[:].rearrange("p (c w) -> p c w", w=W)
                nc.scalar.activation(out=s3[:, 0:gc, 0:f], in_=g3[:, 0:gc, :],
                                     func=AF.Copy)
                nc.scalar.square(out=s3[:, 0:gc, f:2 * f], in_=g3[:, 0:gc, :])
                nc.scalar.activation(out=s3[:, 0:gc, 2 * f:W],
                                     in_=g3[:, 0:gc, 0:1],
                                     func=AF.Copy, bias=1.0, scale=0.0)

                pss = [psump.tile([P, W], F32, tag="ps", name=f"ps_{g}_{bl}")
                       for bl in range(GB)]
                for q in range(NQUART):
                    oh = ohp.tile([P, maxgt * P], MDT)
                    nc.vector.tensor_tensor(
                        out=oh[:, 0:gt * P].rearrange("p (c e) -> p c e", e=P),
                        in0=tg[:, q * gt:(q + 1) * gt]
                            .rearrange("p (c u) -> p c u", u=1)
                            .to_broadcast([P, gt, P]),
                        in1=iotat[:, 0:gt * P]
                            .rearrange("p (c e) -> p c e", e=P),
                        op=AO.is_equal,
                    )
                    toff = 0
                    for bl in range(GB):
                        nt = tiles[g * GB + bl]
                        for t in range(nt):
                            cl = q * gt + toff + t
                            nc.tensor.matmul(
                                out=pss[bl][:],
                                lhsT=oh[:, (toff + t) * P:(toff + t + 1) * P],
                                rhs=sqx[:, cl * W:(cl + 1) * W],
                                start=(q == 0 and t == 0),
                                stop=(q == NQUART - 1 and t == nt - 1),
                            )
                        toff += nt

                # batched finishing pass over the group's GB blocks
                fin = finp.tile([P, GB * W], F32, tag="fin")
                for bl in range(GB):
                    nc.scalar.activation(out=fin[:, bl * W:(bl + 1) * W],
                                         in_=pss[bl][:], func=AF.Copy)
                f3 = fin[:].rearrange("p (b w) -> p b w", w=W)
                cnt = finp.tile([P, GB], F32, tag="cnt")
                nc.vector.tensor_scalar(
                    out=cnt[:].rearrange("p (b u) -> p b u", u=1),
                    in0=f3[:, :, 2 * f:2 * f + 1],
                    scalar1=float(EPS), scalar2=None, op0=AO.max)
                rec = finp.tile([P, GB], F32, tag="rec")
                nc.vector.reciprocal(out=rec[:], in_=cnt[:])
                r3 = rec[:].rearrange("p (b u) -> p b u", u=1)
                mom = finp.tile([P, GB * 2 * f], F32, tag="mom")
                m3 = mom[:].rearrange("p (b w) -> p b w", w=2 * f)
                nc.vector.tensor_tensor(
                    out=m3[:, :, :], in0=f3[:, :, 0:2 * f],
                    in1=r3.to_broadcast([P, GB, 2 * f]), op=AO.mult)
                var = finp.tile([P, GB * f], F32, tag="var")
                v3 = var[:].rearrange("p (b w) -> p b w", w=f)
                nc.vector.tensor_tensor(
                    out=v3[:, :, :], in0=m3[:, :, 0:f], in1=m3[:, :, 0:f],
                    op=AO.mult)
                nc.vector.tensor_tensor(
                    out=v3[:, :, :], in0=m3[:, :, f:2 * f], in1=v3[:, :, :],
                    op=AO.subtract)
                nc.vector.tensor_scalar(
                    out=var[:], in0=var[:], scalar1=0.0, scalar2=None,
                    op0=AO.max)
                std = ovp.tile([P, GB * f], F32, tag="std")
                nc.scalar.sqrt(out=std[:], in_=var[:])
                mask = finp.tile([P, GB], F32, tag="mask")
                nc.vector.tensor_scalar(
                    out=mask[:].rearrange("p (b u) -> p b u", u=1),
                    in0=f3[:, :, 2 * f:2 * f + 1],
                    scalar1=1.5, scalar2=None, op0=AO.is_gt)
                s3o = std[:].rearrange("p (b w) -> p b w", w=f)
                nc.vector.tensor_tensor(
                    out=s3o[:, :, :], in0=s3o[:, :, :],
                    in1=mask[:].rearrange("p (b u) -> p b u", u=1)
                        .to_broadcast([P, GB, f]),
                    op=AO.mult)
                nc.sync.dma_start(
                    out=out3[:, ooff:ooff + GB, :], in_=s3o[:, :, :])

                ioff += 4 * i16g
                coff += gc
                ooff += GB
    return nc


def _balance(deg4):
    """Greedy 4-dim balanced assignment of nodes to NBLK blocks (<=128 each)."""
    tot = deg4.sum(1)
    order = np.argsort(-tot, kind="stable")
    loads = np.zeros((NBLK, NQUART), np.int32)
    cnt = np.zeros(NBLK, np.int32)
    blk = np.empty(N_NODES, np.int64)
    slot = np.empty(N_NODES, np.int64)
    full = np.zeros(NBLK, bool)
    CAP = 512
    for n in order:
        cand = loads + deg4[n]
        mx = cand.max(axis=1)
        sc = np.where((cand > CAP).any(axis=1) | full, np.inf, mx)
        b = int(np.argmin(sc))
        if np.isinf(sc[b]):
            sc2 = np.where(full, np.inf, mx)
            b = int(np.argmin(sc2))
        blk[n] = b
        slot[n] = cnt[b]
        loads[b] += deg4[n]
        cnt[b] += 1
        if cnt[b] >= P:
            full[b] = True
    return blk, slot, loads


def _host_prep(x, edge_index):
    src = np.asarray(edge_index[0], dtype=np.int64)
    tgt = np.asarray(edge_index[1], dtype=np.int64)
    n_edges = src.shape[0]

    eq = src // NQ
    deg4 = np.bincount(tgt * NQUART + eq,
                       minlength=N_NODES * NQUART).reshape(N_NODES, NQUART)
    blk, slot, loads = _balance(deg4.astype(np.int32))

    # sort blocks by max quarter load desc, serpentine-deal to cores so each
    # core's position profile matches; capacity per position = max over cores
    bmax = loads.max(axis=1)
    border = np.argsort(-bmax, kind="stable")    # global block rank
    rank_of = np.empty(NBLK, np.int64)
    rank_of[border] = np.arange(NBLK)
    rounds = rank_of // NCORES
    posn = rank_of % NCORES
    core_of = np.where(rounds % 2 == 0, posn, NCORES - 1 - posn)
    pos_of = rounds                              # block position within core

    # per-position capacity (multiple of 128), same for all cores
    segmax = np.zeros(NB, np.int64)
    np.maximum.at(segmax, pos_of, bmax)
    caps = (np.ceil(np.maximum(segmax, 1) / P).astype(np.int64) * P)

    # per-edge placement
    eb = blk[tgt]
    ecore = core_of[eb]
    epos = pos_of[eb]
    es = slot[tgt]
    # segment id in stream order: (core, group, q, block-in-group)
    egrp = epos // GB
    ebl = epos % GB
    seg = ((ecore * NG + egrp) * NQUART + eq) * GB + ebl
    nseg = NCORES * NG * NQUART * GB
    # capacity per segment id
    segcap = np.empty(nseg, np.int64)
    sid = np.arange(nseg)
    segcap[:] = caps[(sid // (NQUART * GB)) % NG * GB + sid % GB]
    segstart = np.zeros(nseg, np.int64)
    np.cumsum(segcap[:-1], out=segstart[1:])
    total = int(segcap.sum())

    segsums = np.bincount(seg, minlength=nseg)
    assert (segsums <= segcap).all()

    order_e = np.argsort(seg, kind="stable")
    segs = seg[order_e]
    starts = np.zeros(nseg, np.int64)
    np.cumsum(segsums[:-1], out=starts[1:])
    within = np.arange(n_edges) - starts[segs]
    flat = segstart[segs] + within

    gidx_all = np.zeros(total, np.int16)
    tgt_all = np.full(total, -1.0, np.float32)
    gidx_all[flat] = (src[order_e] % NQ).astype(np.int16)
    tgt_all[flat] = es[order_e].astype(np.float32)

    xf = np.ascontiguousarray(np.asarray(x, dtype=np.float32))
    per_core = total // NCORES
    # per (core, g, q): gather stream = GB consecutive segments
    # idx16 wrap: [cap16, 16] -> [16, cap16], replicate x8
    tiles = caps // P
    gtiles = [int(tiles[g * GB:(g + 1) * GB].sum()) for g in range(NG)]

    in_maps = []
    for c in range(NCORES):
        base = c * per_core
        gi_parts = []
        tg_parts = []
        off = base
        for g in range(NG):
            for q in range(NQUART):
                L = gtiles[g] * P
                sidx = gidx_all[off:off + L]
                stgt = tgt_all[off:off + L]
                gi_parts.append(sidx.reshape(L // 16, 16).T)
                tg_parts.append(stgt.reshape(gtiles[g], P).T)
                off += L
        gi16 = np.concatenate(gi_parts, axis=1)          # [16, IC]
        idx16 = np.ascontiguousarray(np.tile(gi16, (8, 1)))
        tcore = np.ascontiguousarray(np.concatenate(tg_parts, axis=1))
        in_maps.append({"x": xf, "gidx": idx16, "tgt": tcore})

    caps_t = tuple(int(v) for v in caps)
    return caps_t, in_maps, core_of[blk], pos_of[blk] * P + slot


def _run(x, edge_index, trace=False):
    from concourse.bass_utils import run_bass_kernel_spmd

    caps_t, in_maps, node_core, node_row = _host_prep(x, edge_index)
    key = ("prog", caps_t, MM_DT)
    if key not in _CACHE:
        nc_ = _build_program(caps_t, MM_DT)
        nc_.finalize()
        _CACHE[key] = nc_
    nc = _CACHE[key]
    res = run_bass_kernel_spmd(
        nc, in_maps, core_ids=list(range(NCORES)), trace=trace)

    outs = [np.asarray(r["out"]) for r in res.results]
    out_full = np.empty((N_NODES, N_FEAT), np.float32)
    for c in range(NCORES):
        m = node_core == c
        out_full[m] = outs[c][node_row[m]]
    return out_full, res


def kernel(**inputs):
    out, _ = _run(inputs["x"], inputs["edge_index"], trace=False)
    return out


# revision 15
# speedup vs baseline: 4.2224x; 1.1497x over previous
"""GNN message-passing (std aggregator) on 8 TRN2 NeuronCores.

Math per target node: count, S1 = sum x[src], S2 = sum x[src]^2;
mean = S1/max(count,eps); var = S2/count - mean^2;
std = sqrt(max(var,0)), zeroed where count <= 1.

Strategy: shard TARGET nodes across cores (no collectives). Host packs nodes
into 128-bin blocks with a greedy 4-dim balancer (per-quarter loads <= ~512),
sorts blocks by load and deals them serpentine to cores so every core has the
same per-position load profile. Each block position gets its own compile-time
capacity (128-multiple), so gather padding is ~2-3% instead of 25%. Per group
of GB blocks and src-quarter q there is ONE dma_gather (int16 idx < 25000);
gathers round-robin 4 SWDGE queues so 4 GpSimd Q7 pairs emit descriptors
concurrently (~3.2x). Per group: ACT builds [x | x^2 | 1] bf16 rhs, DVE builds
one-hot tiles (label-vs-iota is_equal), PE accumulates [128 x 129] = [S1 | S2
| count] per block in PSUM, then a batched finishing pass computes std and one
strided DMA per group writes out.
"""

import numpy as np

N_NODES = 100000
N_FEAT = 64
N_EDGES = 1600000
P = 128
NCORES = 8
NB = 98                 # blocks per core
NBLK = NCORES * NB      # 784
GB = 7                  # blocks per group; 98 = 14*7
NG = NB // GB
NQUART = 4
NQ = N_NODES // NQUART  # rows per src quarter (25000 < 32768 for int16 idx)
EPS = 1e-8
MM_DT = "bfloat16"      # matmul operand dtype

_CACHE = {}


def _build_program(caps, mm_dt):
    """caps: tuple of NB ints, capacity (multiple of 128) per block position."""
    import concourse.bacc as bacc
    import concourse.mybir as mybir
    import concourse.tile as tile

    F32 = mybir.dt.float32
    I16 = mybir.dt.int16
    MDT = getattr(mybir.dt, mm_dt)
    AO = mybir.AluOpType
    AF = mybir.ActivationFunctionType

    f = N_FEAT
    W = 2 * f + 1
    tiles = [c // P for c in caps]               # tile-columns per (pos, q)
    # per-group geometry
    gtiles = [sum(tiles[g * GB:(g + 1) * GB]) for g in range(NG)]  # per q
    gcols_g = [4 * t for t in gtiles]            # tile-cols per group
    maxgt = max(gtiles)
    maxgc = max(gcols_g)
    C = sum(gcols_g)                             # total columns per core
    i16_gq = [t * P // 16 for t in gtiles]       # idx16 cols per (g, q) gather
    IC = 4 * sum(i16_gq)                         # idx16 cols per core

    nc = bacc.Bacc(num_swdge_queues=4)
    xd = nc.declare_dram_parameter("x", [N_NODES, f], F32, isOutput=False)
    gidxd = nc.declare_dram_parameter("gidx", [P, IC], I16, isOutput=False)
    tgtd = nc.declare_dram_parameter("tgt", [P, C], F32, isOutput=False)
    outd = nc.declare_dram_parameter("out", [NB * P, f], F32, isOutput=True)

    with tile.TileContext(nc) as tc:
        with (
            tc.tile_pool(name="const", bufs=1) as constp,
            tc.tile_pool(name="io", bufs=3) as iop,
            tc.tile_pool(name="msg", bufs=2) as msgp,
            tc.tile_pool(name="oh", bufs=2) as ohp,
            tc.tile_pool(name="fin", bufs=2) as finp,
            tc.tile_pool(name="ov", bufs=2) as ovp,
            tc.tile_pool(name="ps", bufs=8, space="PSUM") as psump,
        ):
            iotat = constp.tile([P, maxgt * P], F32)
            nc.gpsimd.iota(iotat[:], pattern=[[0, maxgt], [1, P]], base=0,
                           channel_multiplier=0,
                           allow_small_or_imprecise_dtypes=True)

            out3 = outd[:].rearrange("(b p) f -> p b f", p=P)

            def _drain(pst):
                fin = finp.tile([P, GB * W], F32, tag="fin")
                for j, pt in enumerate(pst):
                    nc.scalar.activation(out=fin[:, j * W:(j + 1) * W],
                                         in_=pt[:], func=AF.Copy)
                return fin

            def _math(fin, ooff):
                f3 = fin[:].rearrange("p (b w) -> p b w", w=W)
                rec = finp.tile([P, GB], F32, tag="rec")
                nc.vector.tensor_scalar(
                    out=rec[:].rearrange("p (b u) -> p b u", u=1),
                    in0=f3[:, :, 2 * f:2 * f + 1],
                    scalar1=float(EPS), scalar2=None, op0=AO.add)
                nc.vector.reciprocal(out=rec[:], in_=rec[:])
                r3 = rec[:].rearrange("p (b u) -> p b u", u=1)
                mom = finp.tile([P, GB * 2 * f], F32, tag="mom")
                m3 = mom[:].rearrange("p (b w) -> p b w", w=2 * f)
                nc.vector.tensor_tensor(
                    out=m3[:, :, :], in0=f3[:, :, 0:2 * f],
                    in1=r3.to_broadcast([P, GB, 2 * f]), op=AO.mult)
                var = finp.tile([P, GB * f], F32, tag="var")
                v3 = var[:].rearrange("p (b w) -> p b w", w=f)
                nc.vector.tensor_tensor(
                    out=v3[:, :, :], in0=m3[:, :, 0:f], in1=m3[:, :, 0:f],
                    op=AO.mult)
                nc.vector.tensor_tensor(
                    out=v3[:, :, :], in0=m3[:, :, f:2 * f], in1=v3[:, :, :],
                    op=AO.subtract)
                std = ovp.tile([P, GB * f], F32, tag="std")
                nc.scalar.activation(out=std[:], in_=var[:], func=AF.Relu)
                nc.scalar.sqrt(out=std[:], in_=std[:])
                mask = finp.tile([P, GB], F32, tag="mask")
                nc.vector.tensor_scalar(
                    out=mask[:].rearrange("p (b u) -> p b u", u=1),
                    in0=f3[:, :, 2 * f:2 * f + 1],
                    scalar1=1.5, scalar2=None, op0=AO.is_gt)
                s3o = std[:].rearrange("p (b w) -> p b w", w=f)
                nc.vector.tensor_tensor(
                    out=s3o[:, :, :], in0=s3o[:, :, :],
                    in1=mask[:].rearrange("p (b u) -> p b u", u=1)
                        .to_broadcast([P, GB, f]),
                    op=AO.mult)
                nc.sync.dma_start(
                    out=out3[:, ooff:ooff + GB, :], in_=s3o[:, :, :])

            pending = []
            ioff = 0   # idx16 column offset
            coff = 0   # tgt column offset
            ooff = 0   # out block offset
            for g in range(NG):
                gt = gtiles[g]
                gc = gcols_g[g]
                i16g = i16_gq[g]
                idx = iop.tile([P, 4 * max(i16_gq)], I16, tag="idx")
                nc.sync.dma_start(out=idx[:, 0:4 * i16g],
                                  in_=gidxd[:, ioff:ioff + 4 * i16g])
                tg = iop.tile([P, maxgc], F32, tag="tg")
                nc.sync.dma_start(out=tg[:, 0:gc],
                                  in_=tgtd[:, coff:coff + gc])

                gbuf = msgp.tile([P, maxgc * f], F32, tag="g")
                g3 = gbuf[:].rearrange("p (c e) -> p c e", e=f)
                for q in range(NQUART):
                    nc.gpsimd.dma_gather(
                        out_ap=g3[:, q * gt:(q + 1) * gt, :],
                        in_ap=xd[q * NQ:(q + 1) * NQ, :],
                        idxs_ap=idx[:, q * i16g:(q + 1) * i16g],
                        num_idxs=gt * P,
                        num_idxs_reg=gt * P,
                        elem_size=f,
                        single_packet=False,
                        queue_num=q,
                    )

                sqx = msgp.tile([P, maxgc * W], MDT, tag="sqx")
                s3 = sqx[:].rearrange("p (c w) -> p c w", w=W)
                for q in range(NQUART):
                    sl = slice(q * gt, (q + 1) * gt)
                    nc.scalar.activation(out=s3[:, sl, 0:f], in_=g3[:, sl, :],
                                         func=AF.Copy)
                    nc.scalar.square(out=s3[:, sl, f:2 * f], in_=g3[:, sl, :])
                    nc.scalar.activation(out=s3[:, sl, 2 * f:W],
                                         in_=g3[:, sl, 0:1],
                                         func=AF.Copy, bias=1.0, scale=0.0)

                pst = [psump.tile([P, W], F32, tag="ps",
                                  name=f"ps_{g}_{j}") for j in range(GB)]
                pss = [pt[:] for pt in pst]
                for q in range(NQUART):
                    oh = ohp.tile([P, maxgt * P], MDT)
                    nc.vector.tensor_tensor(
                        out=oh[:, 0:gt * P].rearrange("p (c e) -> p c e", e=P),
                        in0=tg[:, q * gt:(q + 1) * gt]
                            .rearrange("p (c u) -> p c u", u=1)
                            .to_broadcast([P, gt, P]),
                        in1=iotat[:, 0:gt * P]
                            .rearrange("p (c e) -> p c e", e=P),
                        op=AO.is_equal,
                    )
                    toff = 0
                    for bl in range(GB):
                        nt = tiles[g * GB + bl]
                        for t in range(nt):
                            cl = q * gt + toff + t
                            nc.tensor.matmul(
                                out=pss[bl],
                                lhsT=oh[:, (toff + t) * P:(toff + t + 1) * P],
                                rhs=sqx[:, cl * W:(cl + 1) * W],
                                start=(q == 0 and t == 0),
                                stop=(q == NQUART - 1 and t == nt - 1),
                            )
                        toff += nt

                # finishing deferred one group so its DVE/ACT ops never
                # stall the next group's one-hot builds
                pending.append((_drain(pst), ooff))
                if len(pending) == 2:
                    _math(*pending.pop(0))

                ioff += 4 * i16g
                coff += gc
                ooff += GB
            _math(*pending.pop(0))
    return nc


def _balance(deg4):
    """Greedy 4-dim balanced assignment of nodes to NBLK blocks (<=128 each)."""
    tot = deg4.sum(1)
    order = np.argsort(-tot, kind="stable")
    loads = np.zeros((NBLK, NQUART), np.int32)
    cnt = np.zeros(NBLK, np.int32)
    blk = np.empty(N_NODES, np.int64)
    slot = np.empty(N_NODES, np.int64)
    full = np.zeros(NBLK, bool)
    CAP = 512
    for n in order:
        cand = loads + deg4[n]
        mx = cand.max(axis=1)
        sc = np.where((cand > CAP).any(axis=1) | full, np.inf, mx)
        b = int(np.argmin(sc))
        if np.isinf(sc[b]):
            sc2 = np.where(full, np.inf, mx)
            b = int(np.argmin(sc2))
        blk[n] = b
        slot[n] = cnt[b]
        loads[b] += deg4[n]
        cnt[b] += 1
        if cnt[b] >= P:
            full[b] = True
    return blk, slot, loads


def _host_prep(x, edge_index):
    src = np.asarray(edge_index[0], dtype=np.int64)
    tgt = np.asarray(edge_index[1], dtype=np.int64)
    n_edges = src.shape[0]

    eq = src // NQ
    deg4 = np.bincount(tgt * NQUART + eq,
                       minlength=N_NODES * NQUART).reshape(N_NODES, NQUART)
    blk, slot, loads = _balance(deg4.astype(np.int32))

    # sort blocks by max quarter load desc, serpentine-deal to cores so each
    # core's position profile matches; capacity per position = max over cores
    bmax = loads.max(axis=1)
    border = np.argsort(-bmax, kind="stable")    # global block rank
    rank_of = np.empty(NBLK, np.int64)
    rank_of[border] = np.arange(NBLK)
    rounds = rank_of // NCORES
    posn = rank_of % NCORES
    core_of = np.where(rounds % 2 == 0, posn, NCORES - 1 - posn)
    pos_of = rounds                              # block position within core

    # per-position capacity (multiple of 128), same for all cores
    segmax = np.zeros(NB, np.int64)
    np.maximum.at(segmax, pos_of, bmax)
    caps = (np.ceil(np.maximum(segmax, 1) / P).astype(np.int64) * P)

    # per-edge placement
    eb = blk[tgt]
    ecore = core_of[eb]
    epos = pos_of[eb]
    es = slot[tgt]
    # segment id in stream order: (core, group, q, block-in-group)
    egrp = epos // GB
    ebl = epos % GB
    seg = ((ecore * NG + egrp) * NQUART + eq) * GB + ebl
    nseg = NCORES * NG * NQUART * GB
    # capacity per segment id
    segcap = np.empty(nseg, np.int64)
    sid = np.arange(nseg)
    segcap[:] = caps[(sid // (NQUART * GB)) % NG * GB + sid % GB]
    segstart = np.zeros(nseg, np.int64)
    np.cumsum(segcap[:-1], out=segstart[1:])
    total = int(segcap.sum())

    segsums = np.bincount(seg, minlength=nseg)
    assert (segsums <= segcap).all()

    order_e = np.argsort(seg, kind="stable")
    segs = seg[order_e]
    starts = np.zeros(nseg, np.int64)
    np.cumsum(segsums[:-1], out=starts[1:])
    within = np.arange(n_edges) - starts[segs]
    flat = segstart[segs] + within

    gidx_all = np.zeros(total, np.int16)
    tgt_all = np.full(total, -1.0, np.float32)
    gidx_all[flat] = (src[order_e] % NQ).astype(np.int16)
    tgt_all[flat] = es[order_e].astype(np.float32)

    xf = np.ascontiguousarray(np.asarray(x, dtype=np.float32))
    per_core = total // NCORES
    # per (core, g, q): gather stream = GB consecutive segments
    # idx16 wrap: [cap16, 16] -> [16, cap16], replicate x8
    tiles = caps // P
    gtiles = [int(tiles[g * GB:(g + 1) * GB].sum()) for g in range(NG)]

    in_maps = []
    for c in range(NCORES):
        base = c * per_core
        gi_parts = []
        tg_parts = []
        off = base
        for g in range(NG):
            for q in range(NQUART):
                L = gtiles[g] * P
                sidx = gidx_all[off:off + L]
                stgt = tgt_all[off:off + L]
                gi_parts.append(sidx.reshape(L // 16, 16).T)
                tg_parts.append(stgt.reshape(gtiles[g], P).T)
                off += L
        gi16 = np.concatenate(gi_parts, axis=1)          # [16, IC]
        idx16 = np.ascontiguousarray(np.tile(gi16, (8, 1)))
        tcore = np.ascontiguousarray(np.concatenate(tg_parts, axis=1))
        in_maps.append({"x": xf, "gidx": idx16, "tgt": tcore})

    caps_t = tuple(int(v) for v in caps)
    return caps_t, in_maps, core_of[blk], pos_of[blk] * P + slot


def _run(x, edge_index, trace=False):
    from concourse.bass_utils import run_bass_kernel_spmd

    caps_t, in_maps, node_core, node_row = _host_prep(x, edge_index)
    key = ("prog", caps_t, MM_DT)
    if key not in _CACHE:
        nc_ = _build_program(caps_t, MM_DT)
        nc_.finalize()
        _CACHE[key] = nc_
    nc = _CACHE[key]
    res = run_bass_kernel_spmd(
        nc, in_maps, core_ids=list(range(NCORES)), trace=trace)

    outs = [np.asarray(r["out"]) for r in res.results]
    out_full = np.empty((N_NODES, N_FEAT), np.float32)
    for c in range(NCORES):
        m = node_core == c
        out_full[m] = outs[c][node_row[m]]
    return out_full, res


def kernel(**inputs):
    out, _ = _run(inputs["x"], inputs["edge_index"], trace=False)
    return out


# revision 18
# speedup vs baseline: 4.3435x; 1.0287x over previous
"""GNN message-passing (std aggregator) on 8 TRN2 NeuronCores.

Math per target node: count, S1 = sum x[src], S2 = sum x[src]^2;
mean = S1/max(count,eps); var = S2/count - mean^2;
std = sqrt(max(var,0)), zeroed where count <= 1.

Strategy: shard TARGET nodes across cores (no collectives). Host packs nodes
into 128-bin blocks with a greedy 4-dim balancer (per-quarter loads <= ~512),
sorts blocks by load and deals them serpentine to cores so every core has the
same per-position load profile. Each block position gets its own compile-time
capacity (128-multiple), so gather padding is ~2-3% instead of 25%. Per group
of GB blocks and src-quarter q there is ONE dma_gather (int16 idx < 25000);
gathers round-robin 4 SWDGE queues so 4 GpSimd Q7 pairs emit descriptors
concurrently (~3.2x). Per group: ACT builds [x | x^2 | 1] bf16 rhs, DVE builds
one-hot tiles (label-vs-iota is_equal), PE accumulates [128 x 129] = [S1 | S2
| count] per block in PSUM, then a batched finishing pass computes std and one
strided DMA per group writes out.
"""

import numpy as np

N_NODES = 100000
N_FEAT = 64
N_EDGES = 1600000
P = 128
NCORES = 8
NB = 98                 # blocks per core
NBLK = NCORES * NB      # 784
GB = 7                  # blocks per group; 98 = 14*7
NG = NB // GB
NQUART = 4
NQ = N_NODES // NQUART  # rows per src quarter (25000 < 32768 for int16 idx)
EPS = 1e-8
MM_DT = "bfloat16"      # matmul operand dtype

_CACHE = {}


def _build_program(caps, mm_dt):
    """caps: tuple of NB ints, capacity (multiple of 128) per block position."""
    import concourse.bacc as bacc
    import concourse.mybir as mybir
    import concourse.tile as tile

    F32 = mybir.dt.float32
    I16 = mybir.dt.int16
    MDT = getattr(mybir.dt, mm_dt)
    AO = mybir.AluOpType
    AF = mybir.ActivationFunctionType

    f = N_FEAT
    W = 2 * f + 1
    tiles = [c // P for c in caps]               # tile-columns per (pos, q)
    # per-group geometry
    gtiles = [sum(tiles[g * GB:(g + 1) * GB]) for g in range(NG)]  # per q
    gcols_g = [4 * t for t in gtiles]            # tile-cols per group
    maxgt = max(gtiles)
    maxgc = max(gcols_g)
    C = sum(gcols_g)                             # total columns per core
    i16_gq = [t * P // 16 for t in gtiles]       # idx16 cols per (g, q) gather
    IC = 4 * sum(i16_gq)                         # idx16 cols per core

    nc = bacc.Bacc(num_swdge_queues=4)
    xd = nc.declare_dram_parameter("x", [N_NODES, f], F32, isOutput=False)
    gidxd = nc.declare_dram_parameter("gidx", [P, IC], I16, isOutput=False)
    tgtd = nc.declare_dram_parameter("tgt", [P, C], F32, isOutput=False)
    outd = nc.declare_dram_parameter("out", [NB * P, f], F32, isOutput=True)

    with tile.TileContext(nc) as tc:
        with (
            tc.tile_pool(name="const", bufs=1) as constp,
            tc.tile_pool(name="io", bufs=3) as iop,
            tc.tile_pool(name="msg", bufs=2) as msgp,
            tc.tile_pool(name="oh", bufs=2) as ohp,
            tc.tile_pool(name="fin", bufs=2) as finp,
            tc.tile_pool(name="ov", bufs=2) as ovp,
            tc.tile_pool(name="ps", bufs=8, space="PSUM") as psump,
        ):
            iotat = constp.tile([P, maxgt * P], F32)
            nc.gpsimd.iota(iotat[:], pattern=[[0, maxgt], [1, P]], base=0,
                           channel_multiplier=0,
                           allow_small_or_imprecise_dtypes=True)

            out3 = outd[:].rearrange("(b p) f -> p b f", p=P)

            def _drain(pst):
                fin = finp.tile([P, GB * W], F32, tag="fin")
                for j, pt in enumerate(pst):
                    nc.scalar.activation(out=fin[:, j * W:(j + 1) * W],
                                         in_=pt[:], func=AF.Copy)
                return fin

            def _math(fin, ooff):
                f3 = fin[:].rearrange("p (b w) -> p b w", w=W)
                rec = finp.tile([P, GB], F32, tag="rec")
                nc.vector.tensor_scalar(
                    out=rec[:].rearrange("p (b u) -> p b u", u=1),
                    in0=f3[:, :, 2 * f:2 * f + 1],
                    scalar1=float(EPS), scalar2=None, op0=AO.add)
                nc.vector.reciprocal(out=rec[:], in_=rec[:])
                r3 = rec[:].rearrange("p (b u) -> p b u", u=1)
                mom = finp.tile([P, GB * 2 * f], F32, tag="mom")
                m3 = mom[:].rearrange("p (b w) -> p b w", w=2 * f)
                nc.vector.tensor_tensor(
                    out=m3[:, :, :], in0=f3[:, :, 0:2 * f],
                    in1=r3.to_broadcast([P, GB, 2 * f]), op=AO.mult)
                var = finp.tile([P, GB * f], F32, tag="var")
                v3 = var[:].rearrange("p (b w) -> p b w", w=f)
                nc.vector.tensor_tensor(
                    out=v3[:, :, :], in0=m3[:, :, 0:f], in1=m3[:, :, 0:f],
                    op=AO.mult)
                nc.vector.tensor_tensor(
                    out=v3[:, :, :], in0=m3[:, :, f:2 * f], in1=v3[:, :, :],
                    op=AO.subtract)
                std = ovp.tile([P, GB * f], F32, tag="std")
                nc.scalar.activation(out=std[:], in_=var[:], func=AF.Relu)
                nc.scalar.sqrt(out=std[:], in_=std[:])
                mask = finp.tile([P, GB], F32, tag="mask")
                nc.vector.tensor_scalar(
                    out=mask[:].rearrange("p (b u) -> p b u", u=1),
                    in0=f3[:, :, 2 * f:2 * f + 1],
                    scalar1=1.5, scalar2=None, op0=AO.is_gt)
                s3o = std[:].rearrange("p (b w) -> p b w", w=f)
                nc.vector.tensor_tensor(
                    out=s3o[:, :, :], in0=s3o[:, :, :],
                    in1=mask[:].rearrange("p (b u) -> p b u", u=1)
                        .to_broadcast([P, GB, f]),
                    op=AO.mult)
                nc.sync.dma_start(
                    out=out3[:, ooff:ooff + GB, :], in_=s3o[:, :, :])

            pending = []
            ioff = 0   # idx16 column offset
            coff = 0   # tgt column offset
            ooff = 0   # out block offset
            for g in range(NG):
                gt = gtiles[g]
                gc = gcols_g[g]
                i16g = i16_gq[g]
                idx = iop.tile([P, 4 * max(i16_gq)], I16, tag="idx")
                nc.sync.dma_start(out=idx[:, 0:4 * i16g],
                                  in_=gidxd[:, ioff:ioff + 4 * i16g])
                tg = iop.tile([P, maxgc], F32, tag="tg")
                nc.sync.dma_start(out=tg[:, 0:gc],
                                  in_=tgtd[:, coff:coff + gc])

                if len(pending) == 2:
                    _math(*pending.pop(0))

                gbuf = msgp.tile([P, maxgc * f], F32, tag="g")
                g3 = gbuf[:].rearrange("p (c e) -> p c e", e=f)
                for q in range(NQUART):
                    nc.gpsimd.dma_gather(
                        out_ap=g3[:, q * gt:(q + 1) * gt, :],
                        in_ap=xd[q * NQ:(q + 1) * NQ, :],
                        idxs_ap=idx[:, q * i16g:(q + 1) * i16g],
                        num_idxs=gt * P,
                        num_idxs_reg=gt * P,
                        elem_size=f,
                        single_packet=False,
                        queue_num=q,
                    )

                sqx = msgp.tile([P, maxgc * W], MDT, tag="sqx")
                s3 = sqx[:].rearrange("p (c w) -> p c w", w=W)
                for q in range(NQUART):
                    sl = slice(q * gt, (q + 1) * gt)
                    nc.scalar.activation(out=s3[:, sl, 0:f], in_=g3[:, sl, :],
                                         func=AF.Copy)
                    nc.scalar.square(out=s3[:, sl, f:2 * f], in_=g3[:, sl, :])
                    nc.scalar.activation(out=s3[:, sl, 2 * f:W],
                                         in_=g3[:, sl, 0:1],
                                         func=AF.Copy, bias=1.0, scale=0.0)

                pst = [psump.tile([P, W], F32, tag="ps",
                                  name=f"ps_{g}_{j}") for j in range(GB)]
                pss = [pt[:] for pt in pst]
                for q in range(NQUART):
                    oh = ohp.tile([P, maxgt * P], MDT)
                    nc.vector.tensor_tensor(
                        out=oh[:, 0:gt * P].rearrange("p (c e) -> p c e", e=P),
                        in0=tg[:, q * gt:(q + 1) * gt]
                            .rearrange("p (c u) -> p c u", u=1)
                            .to_broadcast([P, gt, P]),
                        in1=iotat[:, 0:gt * P]
                            .rearrange("p (c e) -> p c e", e=P),
                        op=AO.is_equal,
                    )
                    toff = 0
                    for bl in range(GB):
                        nt = tiles[g * GB + bl]
                        for t in range(nt):
                            cl = q * gt + toff + t
                            nc.tensor.matmul(
                                out=pss[bl],
                                lhsT=oh[:, (toff + t) * P:(toff + t + 1) * P],
                                rhs=sqx[:, cl * W:(cl + 1) * W],
                                start=(q == 0 and t == 0),
                                stop=(q == NQUART - 1 and t == nt - 1),
                            )
                        toff += nt

                # finishing deferred one group so its DVE/ACT ops never
                # stall the next group's one-hot builds
                pending.append((_drain(pst), ooff))

                ioff += 4 * i16g
                coff += gc
                ooff += GB
            while pending:
                _math(*pending.pop(0))
    return nc


def _balance(deg4):
    """Greedy 4-dim balanced assignment of nodes to NBLK blocks (<=128 each)."""
    tot = deg4.sum(1)
    order = np.argsort(-tot, kind="stable")
    loads = np.zeros((NBLK, NQUART), np.int32)
    cnt = np.zeros(NBLK, np.int32)
    blk = np.empty(N_NODES, np.int64)
    slot = np.empty(N_NODES, np.int64)
    full = np.zeros(NBLK, bool)
    CAP = 512
    for n in order:
        cand = loads + deg4[n]
        mx = cand.max(axis=1)
        sc = np.where((cand > CAP).any(axis=1) | full, np.inf, mx)
        b = int(np.argmin(sc))
        if np.isinf(sc[b]):
            sc2 = np.where(full, np.inf, mx)
            b = int(np.argmin(sc2))
        blk[n] = b
        slot[n] = cnt[b]
        loads[b] += deg4[n]
        cnt[b] += 1
        if cnt[b] >= P:
            full[b] = True
    return blk, slot, loads


def _host_prep(x, edge_index):
    src = np.asarray(edge_index[0], dtype=np.int64)
    tgt = np.asarray(edge_index[1], dtype=np.int64)
    n_edges = src.shape[0]

    eq = src // NQ
    deg4 = np.bincount(tgt * NQUART + eq,
                       minlength=N_NODES * NQUART).reshape(N_NODES, NQUART)
    blk, slot, loads = _balance(deg4.astype(np.int32))

    # sort blocks by max quarter load desc, serpentine-deal to cores so each
    # core's position profile matches; capacity per position = max over cores
    bmax = loads.max(axis=1)
    border = np.argsort(-bmax, kind="stable")    # global block rank
    rank_of = np.empty(NBLK, np.int64)
    rank_of[border] = np.arange(NBLK)
    rounds = rank_of // NCORES
    posn = rank_of % NCORES
    core_of = np.where(rounds % 2 == 0, posn, NCORES - 1 - posn)
    pos_of = rounds                              # block position within core

    # per-position capacity (multiple of 128), same for all cores
    segmax = np.zeros(NB, np.int64)
    np.maximum.at(segmax, pos_of, bmax)
    caps = (np.ceil(np.maximum(segmax, 1) / P).astype(np.int64) * P)

    # per-edge placement
    eb = blk[tgt]
    ecore = core_of[eb]
    epos = pos_of[eb]
    es = slot[tgt]
    # segment id in stream order: (core, group, q, block-in-group)
    egrp = epos // GB
    ebl = epos % GB
    seg = ((ecore * NG + egrp) * NQUART + eq) * GB + ebl
    nseg = NCORES * NG * NQUART * GB
    # capacity per segment id
    segcap = np.empty(nseg, np.int64)
    sid = np.arange(nseg)
    segcap[:] = caps[(sid // (NQUART * GB)) % NG * GB + sid % GB]
    segstart = np.zeros(nseg, np.int64)
    np.cumsum(segcap[:-1], out=segstart[1:])
    total = int(segcap.sum())

    segsums = np.bincount(seg, minlength=nseg)
    assert (segsums <= segcap).all()

    order_e = np.argsort(seg, kind="stable")
    segs = seg[order_e]
    starts = np.zeros(nseg, np.int64)
    np.cumsum(segsums[:-1], out=starts[1:])
    within = np.arange(n_edges) - starts[segs]
    flat = segstart[segs] + within

    gidx_all = np.zeros(total, np.int16)
    tgt_all = np.full(total, -1.0, np.float32)
    gidx_all[flat] = (src[order_e] % NQ).astype(np.int16)
    tgt_all[flat] = es[order_e].astype(np.float32)

    xf = np.ascontiguousarray(np.asarray(x, dtype=np.float32))
    per_core = total // NCORES
    # per (core, g, q): gather stream = GB consecutive segments
    # idx16 wrap: [cap16, 16] -> [16, cap16], replicate x8
    tiles = caps // P
    gtiles = [int(tiles[g * GB:(g + 1) * GB].sum()) for g in range(NG)]

    in_maps = []
    for c in range(NCORES):
        base = c * per_core
        gi_parts = []
        tg_parts = []
        off = base
        for g in range(NG):
            for q in range(NQUART):
                L = gtiles[g] * P
                sidx = gidx_all[off:off + L]
                stgt = tgt_all[off:off + L]
                gi_parts.append(sidx.reshape(L // 16, 16).T)
                tg_parts.append(stgt.reshape(gtiles[g], P).T)
                off += L
        gi16 = np.concatenate(gi_parts, axis=1)          # [16, IC]
        idx16 = np.ascontiguousarray(np.tile(gi16, (8, 1)))
        tcore = np.ascontiguousarray(np.concatenate(tg_parts, axis=1))
        in_maps.append({"x": xf, "gidx": idx16, "tgt": tcore})

    caps_t = tuple(int(v) for v in caps)
    return caps_t, in_maps, core_of[blk], pos_of[blk] * P + slot


def _run(x, edge_index, trace=False):
    from concourse.bass_utils import run_bass_kernel_spmd

    caps_t, in_maps, node_core, node_row = _host_prep(x, edge_index)
    key = ("prog", caps_t, MM_DT)
    if key not in _CACHE:
        nc_ = _build_program(caps_t, MM_DT)
        nc_.finalize()
        _CACHE[key] = nc_
    nc = _CACHE[key]
    res = run_bass_kernel_spmd(
        nc, in_maps, core_ids=list(range(NCORES)), trace=trace)

    outs = [np.asarray(r["out"]) for r in res.results]
    out_full = np.empty((N_NODES, N_FEAT), np.float32)
    for c in range(NCORES):
        m = node_core == c
        out_full[m] = outs[c][node_row[m]]
    return out_full, res


def kernel(**inputs):
    out, _ = _run(inputs["x"], inputs["edge_index"], trace=False)
    return out
